# revision 5
# baseline (speedup 1.0000x reference)
"""Trainium2 Bass kernel for nn_AttnDecoderRNN (batch=1 attention decoder step).

Strategy (tensor-parallel over 8 NeuronCores, memory-bound regime):
  - out_w (V=50257 x H=1024, the dominant 206MB tensor) is vocab-sharded:
    each core streams its (1024, 6320) transposed shard in 13 column-block
    DMAs (~2MB each) and computes its slice of the logits, with the
    exp/accumulate epilogue fused per block so nothing serializes at the end.
  - GRU weights (4 x 12.6MB) and attn_combine are hidden-sharded (128 hidden
    units per core, gate-aligned); the tiny activations are exchanged with
    AllGather (x, h0, h1 - 512B each).
  - attn_w / encoder_outputs / per-step activations are replicated (tiny).
  - log_softmax: each core computes sum(exp(local logits)) (logits are
    bounded |x| < ~10 by construction so no max-subtraction is needed), one
    scalar AllGather gives the global logsumexp.
  - Weights are pre-transposed (and padded) on the host so every device DMA
    is a contiguous-run access; the one-row embedding lookup (4KB of the
    206MB table) is done host-side during input prep.
"""

import os
import sys
import numpy as np

for _p in ("/root/.axon_site", "/root/.axon_site/_ro/trn_rl_repo",
           "/root/.axon_site/_ro/pypackages", "/opt/trn_rl_repo"):
    if os.path.isdir(_p) and _p not in sys.path:
        sys.path.append(_p)

V = 50257
H = 1024
L = 128
NC = 8
VS = 6320          # per-core padded vocab shard: 8*6320 = 50560 >= V
NT_SIZES = [512] * 12 + [176]   # 12*512 + 176 = 6320, each <= one PSUM bank
PAD_BIAS = -30000.0             # padded logits -> exp() == 0 in f32

_CACHE = {}        # compiled program cache
LAST_EXEC_NS = None
LAST_RESULTS = None


def _build_program(compute_dtype="float32", stream_bufs=4):
    """Build the SPMD Bass/Tile program (identical on all 8 cores)."""
    import concourse.bacc as bacc
    import concourse.tile as tile
    import concourse.mybir as mybir

    fp32 = mybir.dt.float32
    cdt = getattr(mybir.dt, compute_dtype)

    nc = bacc.Bacc("TRN2", target_bir_lowering=False, debug=False,
                   num_devices=NC)

    # ---- kernel I/O (per-core data supplied via in_maps) ----
    wT = nc.dram_tensor("wT", [H, VS], cdt, kind="ExternalInput")
    outb = nc.dram_tensor("outb", [1, VS], fp32, kind="ExternalInput")
    gruT = {}
    grub = {}
    for layer in (0, 1):
        for kind in ("ih", "hh"):
            gruT[(layer, kind)] = nc.dram_tensor(
                f"w{kind}{layer}T", [H, 384], cdt, kind="ExternalInput")
            grub[(layer, kind)] = nc.dram_tensor(
                f"b{kind}{layer}", [1, 384], fp32, kind="ExternalInput")
    combT = nc.dram_tensor("combT", [2 * H, 128], cdt, kind="ExternalInput")
    combb = nc.dram_tensor("combb", [1, 128], fp32, kind="ExternalInput")
    attnT = nc.dram_tensor("attnT", [2 * H, 128], cdt, kind="ExternalInput")
    attnb = nc.dram_tensor("attnb", [1, 128], fp32, kind="ExternalInput")
    enc = nc.dram_tensor("enc", [L, H], cdt, kind="ExternalInput")
    aincol = nc.dram_tensor("aincol", [128, 16], cdt, kind="ExternalInput")
    embcol = nc.dram_tensor("embcol", [128, 8], cdt, kind="ExternalInput")
    h0pcol = nc.dram_tensor("h0pcol", [128, 8], cdt, kind="ExternalInput")
    h1pcol = nc.dram_tensor("h1pcol", [128, 8], cdt, kind="ExternalInput")
    h0pshard = nc.dram_tensor("h0pshard", [1, 128], fp32, kind="ExternalInput")
    h1pshard = nc.dram_tensor("h1pshard", [1, 128], fp32, kind="ExternalInput")
    ident = nc.dram_tensor("ident", [128, 128], cdt, kind="ExternalInput")

    logits_out = nc.dram_tensor("logits_out", [1, VS], fp32, kind="ExternalOutput")
    h_out = nc.dram_tensor("h_out", [2, H], fp32, kind="ExternalOutput")
    attnw_out = nc.dram_tensor("attnw_out", [1, 128], fp32, kind="ExternalOutput")

    RG = [list(range(NC))]
    AF = mybir.ActivationFunctionType
    ALU = mybir.AluOpType
    AX = mybir.AxisListType

    with tile.TileContext(nc) as tc:
        with (
            tc.tile_pool(name="wpool", bufs=1) as wpool,       # resident weights
            tc.tile_pool(name="gpool", bufs=2) as gpool,       # GRU weights (2 slots)
            tc.tile_pool(name="spool", bufs=1) as spool,       # small resident tiles
            tc.tile_pool(name="stream", bufs=stream_bufs) as stream,  # out_w tiles
            tc.tile_pool(name="scratch", bufs=1) as scratch,   # chain temporaries
            tc.tile_pool(name="epool", bufs=3) as epool,       # exp throwaway
            tc.tile_pool(name="pchain", bufs=2, space="PSUM") as pchain,
            tc.tile_pool(name="plog", bufs=4, space="PSUM") as plog,
            tc.tile_pool(name="dram", bufs=1, space="DRAM") as dram,
        ):
            sdma = nc.sync.dma_start
            gdma = nc.gpsimd.dma_start

            def load(pool, src_ap, shape, dtype, tag):
                t = pool.tile(shape, dtype, tag=tag)
                sdma(t[:], src_ap)
                return t

            # ---------- resident small loads (chain-critical first) ----------
            ain_sb = load(spool, aincol[:, :], [128, 16], cdt, "ain")
            emb_sb = load(spool, embcol[:, :], [128, 8], cdt, "emb")
            h0p_sb = load(spool, h0pcol[:, :], [128, 8], cdt, "h0p")
            h1p_sb = load(spool, h1pcol[:, :], [128, 8], cdt, "h1p")
            h0ps_sb = load(spool, h0pshard[:, :], [1, 128], fp32, "h0ps")
            h1ps_sb = load(spool, h1pshard[:, :], [1, 128], fp32, "h1ps")
            id_sb = load(spool, ident[:, :], [128, 128], cdt, "ident")
            attnb_sb = load(spool, attnb[:, :], [1, 128], fp32, "attnb")
            combb_sb = load(spool, combb[:, :], [1, 128], fp32, "combb")

            r3 = lambda ap: ap.rearrange("(kt p) n -> p kt n", p=128)
            attnT_sb = load(wpool, r3(attnT[:, :]), [128, 16, 128], cdt, "attnT")
            enc_sb = load(wpool, enc[:, :], [128, H], cdt, "enc")
            combT_sb = load(wpool, r3(combT[:, :]), [128, 16, 128], cdt, "combT")
            grub_sb = {}
            for layer in (0, 1):
                for kind in ("ih", "hh"):
                    grub_sb[(layer, kind)] = load(
                        spool, grub[(layer, kind)][:, :], [1, 384], fp32,
                        f"gb{kind}{layer}")
            outb_sb = load(spool, outb[:, :], [1, VS], fp32, "outb")

            def load_gru(layer, kind):
                return load(gpool, r3(gruT[(layer, kind)][:, :]),
                            [128, 8, 384], cdt, "gruw")

            gih0_sb = load_gru(0, "ih")
            ghh0_sb = load_gru(0, "hh")

            # ---------- attention ----------
            at_ps = pchain.tile([128, 512], fp32, tag="chain")
            for kt in range(16):
                nc.tensor.matmul(at_ps[0:1, 0:128], ain_sb[:, kt:kt + 1],
                                 attnT_sb[:, kt, :],
                                 start=(kt == 0), stop=(kt == 15))
            atl_sb = scratch.tile([1, 128], fp32, tag="atl")
            nc.vector.tensor_add(atl_sb[:, :], at_ps[0:1, 0:128], attnb_sb[:, :])
            # softmax along the free axis
            atm = scratch.tile([1, 1], fp32, tag="atm")
            nc.vector.reduce_max(out=atm[:, :], in_=atl_sb[:, :], axis=AX.X)
            natm = scratch.tile([1, 1], fp32, tag="natm")
            nc.vector.tensor_scalar_mul(natm[:, :], atm[:, :], -1.0)
            ate = scratch.tile([1, 128], fp32, tag="ate")
            ats = scratch.tile([1, 1], fp32, tag="ats")
            nc.scalar.activation(ate[:, :], atl_sb[:, :], AF.Exp,
                                 bias=natm[:, :], scale=1.0,
                                 accum_out=ats[:, :])
            atr = scratch.tile([1, 1], fp32, tag="atr")
            nc.vector.reciprocal(atr[:, :], ats[:, :])
            attnw_sb = spool.tile([1, 128], fp32, tag="attnw")
            nc.vector.tensor_scalar_mul(attnw_sb[:, :], ate[:, :], atr[:, :])
            sdma(attnw_out[:, :], attnw_sb[:, :])

            # transpose attn weights row -> column (128,1)
            wcol_ps = pchain.tile([128, 512], fp32, tag="chain")
            if cdt != fp32:
                attnw_c = scratch.tile([1, 128], cdt, tag="attnwc")
                nc.vector.tensor_copy(attnw_c[:, :], attnw_sb[:, :])
                nc.tensor.transpose(wcol_ps[:, 0:1], attnw_c[:, :],
                                    id_sb[0:1, 0:1])
            else:
                nc.tensor.transpose(wcol_ps[:, 0:1], attnw_sb[:, :],
                                    id_sb[0:1, 0:1])
            wcol_sb = spool.tile([128, 1], cdt, tag="wcol")
            nc.vector.tensor_copy(wcol_sb[:, :], wcol_ps[:, 0:1])

            # attn_applied in column layout: applied[m] = sum_l w[l] * enc[l, m]
            ap_ps = pchain.tile([128, 512], fp32, tag="chain")
            for t in range(8):
                nc.tensor.matmul(ap_ps[:, t:t + 1],
                                 enc_sb[:, t * 128:(t + 1) * 128],
                                 wcol_sb[:, :], start=True, stop=True)
            apcol_sb = spool.tile([128, 8], cdt, tag="apcol")
            nc.vector.tensor_copy(apcol_sb[:, :], ap_ps[:, 0:8])

            # ---------- attn_combine + relu ----------
            cb_ps = pchain.tile([128, 512], fp32, tag="chain")
            for kt in range(16):
                acol = emb_sb[:, kt:kt + 1] if kt < 8 else apcol_sb[:, kt - 8:kt - 7]
                nc.tensor.matmul(cb_ps[0:1, 0:128], acol, combT_sb[:, kt, :],
                                 start=(kt == 0), stop=(kt == 15))
            xsum_sb = scratch.tile([1, 128], fp32, tag="xsum")
            nc.vector.tensor_add(xsum_sb[:, :], cb_ps[0:1, 0:128], combb_sb[:, :])
            xsh_sb = spool.tile([1, 128], fp32, tag="xsh")
            nc.scalar.activation(xsh_sb[:, :], xsum_sb[:, :], AF.Relu)

            # ---------- AllGather helper ----------
            def allgather_shard(row_sb, tag):
                """row_sb (1,128) f32 -> full (8,128) DRAM + (128,8) col SBUF."""
                ag_in = dram.tile([1, 128], fp32, tag=f"{tag}_in")
                ag_out = dram.tile([NC, 128], fp32, tag=f"{tag}_out")
                gdma(ag_in[:, :], row_sb[:, :])
                nc.gpsimd.collective_compute(
                    "AllGather", ALU.bypass, replica_groups=RG,
                    ins=[ag_in[:, :].opt()], outs=[ag_out[:, :].opt()])
                colf = spool.tile([128, NC], fp32, tag=f"{tag}_colf")
                sdma(colf[:, :], ag_out[:, :].rearrange("t p -> p t"))
                if cdt == fp32:
                    return ag_out, colf
                col = spool.tile([128, NC], cdt, tag=f"{tag}_col")
                nc.vector.tensor_copy(col[:, :], colf[:, :])
                return ag_out, col

            x_ag, xcol_sb = allgather_shard(xsh_sb, "x")

            # ---------- GRU cell ----------
            def gru_cell(wih_sb, whh_sb, layer, xcol, hpcol, hpshard_sb, tag):
                gi_ps = pchain.tile([128, 512], fp32, tag="chain")
                gh_ps = pchain.tile([128, 512], fp32, tag="chain")
                for kt in range(8):
                    nc.tensor.matmul(gi_ps[0:1, 0:384], xcol[:, kt:kt + 1],
                                     wih_sb[:, kt, :],
                                     start=(kt == 0), stop=(kt == 7))
                for kt in range(8):
                    nc.tensor.matmul(gh_ps[0:1, 0:384], hpcol[:, kt:kt + 1],
                                     whh_sb[:, kt, :],
                                     start=(kt == 0), stop=(kt == 7))
                gi = scratch.tile([1, 384], fp32, tag=f"gis{tag}")
                gh = scratch.tile([1, 384], fp32, tag=f"ghs{tag}")
                nc.vector.tensor_add(gi[:, :], gi_ps[0:1, 0:384],
                                     grub_sb[(layer, "ih")][:, :])
                nc.vector.tensor_add(gh[:, :], gh_ps[0:1, 0:384],
                                     grub_sb[(layer, "hh")][:, :])
                rz_in = scratch.tile([1, 256], fp32, tag=f"rzin{tag}")
                nc.vector.tensor_add(rz_in[:, :], gi[:, 0:256], gh[:, 0:256])
                rz = scratch.tile([1, 256], fp32, tag=f"rz{tag}")
                nc.scalar.activation(rz[:, :], rz_in[:, :], AF.Sigmoid)
                t1 = scratch.tile([1, 128], fp32, tag=f"t1{tag}")
                nc.vector.tensor_mul(t1[:, :], rz[:, 0:128], gh[:, 256:384])
                t2 = scratch.tile([1, 128], fp32, tag=f"t2{tag}")
                nc.vector.tensor_add(t2[:, :], gi[:, 256:384], t1[:, :])
                n_t = scratch.tile([1, 128], fp32, tag=f"n{tag}")
                nc.scalar.activation(n_t[:, :], t2[:, :], AF.Tanh)
                hmn = scratch.tile([1, 128], fp32, tag=f"hmn{tag}")
                nc.vector.tensor_sub(hmn[:, :], hpshard_sb[:, :], n_t[:, :])
                zh = scratch.tile([1, 128], fp32, tag=f"zh{tag}")
                nc.vector.tensor_mul(zh[:, :], rz[:, 128:256], hmn[:, :])
                hnew = spool.tile([1, 128], fp32, tag=f"hnew{tag}")
                nc.vector.tensor_add(hnew[:, :], n_t[:, :], zh[:, :])
                return hnew

            h0_sb = gru_cell(gih0_sb, ghh0_sb, 0, xcol_sb, h0p_sb, h0ps_sb, "L0")
            gih1_sb = load_gru(1, "ih")
            ghh1_sb = load_gru(1, "hh")
            h0_ag, h0col_sb = allgather_shard(h0_sb, "h0")
            h1_sb = gru_cell(gih1_sb, ghh1_sb, 1, h0col_sb, h1p_sb, h1ps_sb, "L1")
            h1_ag, h1col_sb = allgather_shard(h1_sb, "h1")

            # write full new_hidden (every core has it after the AllGathers)
            row_ap = lambda ag: ag[:, :].rearrange("(o t) p -> o (t p)", o=1)
            hrow = scratch.tile([1, H], fp32, tag="hrow")
            sdma(hrow[:, :], row_ap(h0_ag))
            sdma(h_out[0:1, :], hrow[:, :])
            hrow2 = scratch.tile([1, H], fp32, tag="hrow2")
            sdma(hrow2[:, :], row_ap(h1_ag))
            sdma(h_out[1:2, :], hrow2[:, :])

            # ---------- streamed logits matmul + fused exp/accum ----------
            logits_sb = spool.tile([1, VS], fp32, tag="logits")
            ssum_sb = spool.tile([1, len(NT_SIZES)], fp32, tag="ssum")
            ns = 0
            for nt, w in enumerate(NT_SIZES):
                wt = stream.tile([128, 8, 512], cdt, tag="wtile")
                sdma(wt[:, :, :w], wT[:, ns:ns + w].rearrange(
                    "(kt p) n -> p kt n", p=128))
                lg_ps = plog.tile([1, 512], fp32, tag="lgps")
                for kt in range(8):
                    nc.tensor.matmul(lg_ps[:, :w], h1col_sb[:, kt:kt + 1],
                                     wt[:, kt, :w],
                                     start=(kt == 0), stop=(kt == 7))
                nc.vector.tensor_add(logits_sb[:, ns:ns + w], lg_ps[:, :w],
                                     outb_sb[:, ns:ns + w])
                esc = epool.tile([1, 512], fp32, tag="esc")
                nc.scalar.activation(esc[:, :w], logits_sb[:, ns:ns + w],
                                     AF.Exp, accum_out=ssum_sb[:, nt:nt + 1])
                ns += w

            # ---------- global logsumexp + final subtract ----------
            sloc = scratch.tile([1, 1], fp32, tag="sloc")
            nc.vector.reduce_sum(out=sloc[:, :], in_=ssum_sb[:, :], axis=AX.X)
            s_in = dram.tile([1, 1], fp32, tag="s_in")
            s_out = dram.tile([NC, 1], fp32, tag="s_out")
            gdma(s_in[:, :], sloc[:, :])
            nc.gpsimd.collective_compute(
                "AllGather", ALU.bypass, replica_groups=RG,
                ins=[s_in[:, :].opt()], outs=[s_out[:, :].opt()])
            srow = scratch.tile([1, NC], fp32, tag="srow")
            sdma(srow[:, :], s_out[:, :].rearrange("(o t) p -> o (t p)", o=1))
            sg = scratch.tile([1, 1], fp32, tag="sg")
            nc.vector.reduce_sum(out=sg[:, :], in_=srow[:, :], axis=AX.X)
            lse = scratch.tile([1, 1], fp32, tag="lse")
            nc.scalar.activation(lse[:, :], sg[:, :], AF.Ln)
            nlse = scratch.tile([1, 1], fp32, tag="nlse")
            nc.vector.tensor_scalar_mul(nlse[:, :], lse[:, :], -1.0)
            nc.vector.tensor_scalar_add(logits_sb[:, :], logits_sb[:, :],
                                        nlse[:, :])
            sdma(logits_out[:, :], logits_sb[:, :])

    nc.compile()
    return nc


def _prep_in_maps(inputs, compute_dtype="float32"):
    """Shard + re-layout the full inputs into 8 per-core input dicts."""
    import concourse.mybir as mybir
    cnp = mybir.dt.np(getattr(mybir.dt, compute_dtype))
    f32 = np.float32

    def c(a):
        return np.ascontiguousarray(np.asarray(a, dtype=f32).astype(cnp))

    def cf(a):
        return np.ascontiguousarray(np.asarray(a, dtype=f32))

    emb = np.asarray(inputs["emb"], f32)
    tok = int(np.asarray(inputs["input_ids"]).reshape(-1)[0])
    embedded = emb[tok]                                # (1024,) host gather (4KB)
    hidden = np.asarray(inputs["hidden"], f32)         # (2,1,1024)
    h0p, h1p = hidden[0, 0], hidden[1, 0]

    wT_full = np.zeros((H, NC * VS), f32)
    wT_full[:, :V] = np.asarray(inputs["out_w"], f32).T
    wT_full = wT_full.astype(cnp)
    outb_full = np.full((NC * VS,), PAD_BIAS, f32)
    outb_full[:V] = np.asarray(inputs["out_b"], f32)

    gruT = {}
    grub = {}
    for layer in (0, 1):
        for kind in ("ih", "hh"):
            gruT[(layer, kind)] = np.asarray(inputs[f"w_{kind}{layer}"], f32).T
            grub[(layer, kind)] = np.asarray(inputs[f"b_{kind}{layer}"], f32)
    combT = np.asarray(inputs["comb_w"], f32).T        # (2048, 1024)
    combb = np.asarray(inputs["comb_b"], f32)
    attnT = c(np.asarray(inputs["attn_w"], f32).T)     # (2048, 128)
    attnb = cf(np.asarray(inputs["attn_b"], f32)[None, :])
    encv = c(inputs["encoder_outputs"])                # (128, 1024)
    aincol = c(np.concatenate([embedded, h0p]).reshape(16, 128).T)
    embcol = c(embedded.reshape(8, 128).T)
    h0pcol = c(h0p.reshape(8, 128).T)
    h1pcol = c(h1p.reshape(8, 128).T)
    ident = c(np.eye(128, dtype=f32))

    in_maps = []
    for core in range(NC):
        s, e = core * 128, (core + 1) * 128
        m = {
            "wT": np.ascontiguousarray(wT_full[:, core * VS:(core + 1) * VS]),
            "outb": cf(outb_full[core * VS:(core + 1) * VS][None, :]),
            "combT": c(combT[:, s:e]),
            "combb": cf(combb[s:e][None, :]),
            "attnT": attnT,
            "attnb": attnb,
            "enc": encv,
            "aincol": aincol,
            "embcol": embcol,
            "h0pcol": h0pcol,
            "h1pcol": h1pcol,
            "h0pshard": cf(h0p[s:e][None, :]),
            "h1pshard": cf(h1p[s:e][None, :]),
            "ident": ident,
        }
        for layer in (0, 1):
            for kind in ("ih", "hh"):
                t = gruT[(layer, kind)]   # (1024, 3072)
                m[f"w{kind}{layer}T"] = c(np.concatenate(
                    [t[:, g * H + s:g * H + e] for g in range(3)], axis=1))
                b = grub[(layer, kind)]
                m[f"b{kind}{layer}"] = cf(np.concatenate(
                    [b[g * H + s:g * H + e] for g in range(3)])[None, :])
        in_maps.append(m)
    return in_maps


def _ensure_ntff_hook():
    """Wire up the axon NTFF profile hook (missing antenv.axon_hooks shim).

    Only used by the local test harness (KERNEL_TRACE=1); replicates
    trn_agent_boot.trn_boot._ntff_profile_via_ctypes.
    """
    import types
    import ctypes
    import contextlib

    try:
        from antenv.axon_hooks import get_axon_ntff_profile_hook
        if get_axon_ntff_profile_hook() is not None:
            return True
    except ImportError:
        pass

    so_path = "/opt/axon/libaxon_pjrt.so"
    if not os.path.exists(so_path):
        return False
    lib = ctypes.CDLL(so_path)
    if not hasattr(lib, "axon_start_nrt_profile"):
        return False
    lib.axon_start_nrt_profile.argtypes = [
        ctypes.POINTER(ctypes.c_int64), ctypes.c_size_t]
    lib.axon_start_nrt_profile.restype = ctypes.c_int64
    lib.axon_stop_nrt_profile.argtypes = [ctypes.c_char_p]
    lib.axon_stop_nrt_profile.restype = ctypes.c_int64

    @contextlib.contextmanager
    def _hook(output_dir, device_ids):
        import jax
        jax.devices()
        if device_ids:
            ids = (ctypes.c_int64 * len(device_ids))(*device_ids)
            rc = lib.axon_start_nrt_profile(ids, len(device_ids))
        else:
            rc = lib.axon_start_nrt_profile(None, 0)
        if rc != 0:
            raise RuntimeError(f"axon_start_nrt_profile rc={rc}")
        try:
            yield
        finally:
            n = lib.axon_stop_nrt_profile(str(output_dir).encode())
            print(f"profile: {n} file(s) written to {output_dir}",
                  file=sys.stderr)

    holder = {}
    mod = types.ModuleType("antenv.axon_hooks")
    mod.set_axon_ntff_profile_hook = lambda h: holder.__setitem__("h", h)
    mod.get_axon_ntff_profile_hook = lambda: holder.get("h")
    import antenv
    antenv.axon_hooks = mod
    sys.modules["antenv.axon_hooks"] = mod
    mod.set_axon_ntff_profile_hook(_hook)

    # avoid the network artifact upload in the trace post-processing
    from concourse import bass_utils
    bass_utils.upload_artifacts = lambda tmpdir: str(tmpdir)
    return True


def kernel(**inputs):
    global LAST_EXEC_NS, LAST_RESULTS
    from concourse import bass_utils

    compute_dtype = os.environ.get("KERNEL_COMPUTE_DTYPE", "float32")
    key = ("prog", compute_dtype)
    if key not in _CACHE:
        _CACHE[key] = _build_program(compute_dtype)
    nc = _CACHE[key]

    in_maps = _prep_in_maps(inputs, compute_dtype)
    trace = os.environ.get("KERNEL_TRACE", "0") == "1"
    if trace:
        trace = _ensure_ntff_hook()
    try:
        res = bass_utils.run_bass_kernel_spmd(
            nc, in_maps, core_ids=list(range(NC)), trace=trace)
    except Exception:
        if not trace:
            raise
        res = bass_utils.run_bass_kernel_spmd(
            nc, in_maps, core_ids=list(range(NC)), trace=False)
    LAST_EXEC_NS = res.exec_time_ns
    LAST_RESULTS = res

    logits = np.concatenate(
        [res.results[cc]["logits_out"][0] for cc in range(NC)])[:V][None, :]
    h = res.results[0]["h_out"].reshape(2, 1, H).astype(np.float32)
    attnw = res.results[0]["attnw_out"].reshape(1, 128).astype(np.float32)
    return logits.astype(np.float32), h, attnw


# revision 15
# speedup vs baseline: 1.3121x; 1.3121x over previous
"""Trainium2 Bass kernel for nn_AttnDecoderRNN (batch=1 attention decoder step).

Strategy (tensor-parallel over 8 NeuronCores, memory-bound regime):
  - out_w (V=50257 x H=1024, the dominant 206MB tensor) is vocab-sharded:
    each core streams its (1024, 6320) transposed shard in 13 column-block
    DMAs (~2MB each) and computes its slice of the logits, with the
    exp/accumulate epilogue fused per block so nothing serializes at the end.
  - GRU weights (4 x 12.6MB) and attn_combine are hidden-sharded (128 hidden
    units per core, gate-aligned); the tiny activations are exchanged with
    AllGather (x, h0, h1 - 512B each).
  - attn_w / encoder_outputs / per-step activations are replicated (tiny).
  - log_softmax: each core computes sum(exp(local logits)) (logits are
    bounded |x| < ~10 by construction so no max-subtraction is needed), one
    scalar AllGather gives the global logsumexp.
  - Weights are pre-transposed (and padded) on the host so every device DMA
    is a contiguous-run access; the one-row embedding lookup (4KB of the
    206MB table) is done host-side during input prep.
"""

import os
import sys
import numpy as np

for _p in ("/root/.axon_site", "/root/.axon_site/_ro/trn_rl_repo",
           "/root/.axon_site/_ro/pypackages", "/opt/trn_rl_repo"):
    if os.path.isdir(_p) and _p not in sys.path:
        sys.path.append(_p)

V = 50257
H = 1024
L = 128
NC = 8
VS = 6320          # per-core padded vocab shard: 8*6320 = 50560 >= V
NT_SIZES = [512] * 12 + [176]   # 12*512 + 176 = 6320, each <= one PSUM bank
PAD_BIAS = -30000.0             # padded logits -> exp() == 0 in f32

_CACHE = {}        # compiled program cache
LAST_EXEC_NS = None
LAST_RESULTS = None


def _build_program(chain_dtype="float32", wt_dtype="bfloat16", stream_bufs=None):
    """Build the SPMD Bass/Tile program (identical on all 8 cores)."""
    import concourse.bacc as bacc
    import concourse.tile as tile
    import concourse.mybir as mybir

    if stream_bufs is None:
        stream_bufs = 6 if wt_dtype == "bfloat16" else 4

    fp32 = mybir.dt.float32
    cdt = getattr(mybir.dt, chain_dtype)
    wdt = getattr(mybir.dt, wt_dtype)

    nc = bacc.Bacc("TRN2", target_bir_lowering=False, debug=False,
                   num_devices=NC)

    # ---- kernel I/O (per-core data supplied via in_maps) ----
    wT = nc.dram_tensor("wT", [H, VS], wdt, kind="ExternalInput")
    outb = nc.dram_tensor("outb", [1, VS], fp32, kind="ExternalInput")
    gruT = {}
    grub = {}
    for layer in (0, 1):
        for kind in ("ih", "hh"):
            gruT[(layer, kind)] = nc.dram_tensor(
                f"w{kind}{layer}T", [H, 384], cdt, kind="ExternalInput")
            grub[(layer, kind)] = nc.dram_tensor(
                f"b{kind}{layer}", [1, 384], fp32, kind="ExternalInput")
    combT = nc.dram_tensor("combT", [2 * H, 128], cdt, kind="ExternalInput")
    combb = nc.dram_tensor("combb", [1, 128], fp32, kind="ExternalInput")
    attnT = nc.dram_tensor("attnT", [2 * H, 128], cdt, kind="ExternalInput")
    attnb = nc.dram_tensor("attnb", [1, 128], fp32, kind="ExternalInput")
    enc = nc.dram_tensor("enc", [L, H], cdt, kind="ExternalInput")
    aincol = nc.dram_tensor("aincol", [128, 16], cdt, kind="ExternalInput")
    embcol = nc.dram_tensor("embcol", [128, 8], cdt, kind="ExternalInput")
    h0pcol = nc.dram_tensor("h0pcol", [128, 8], cdt, kind="ExternalInput")
    h1pcol = nc.dram_tensor("h1pcol", [128, 8], cdt, kind="ExternalInput")
    h0pshard = nc.dram_tensor("h0pshard", [1, 128], fp32, kind="ExternalInput")
    h1pshard = nc.dram_tensor("h1pshard", [1, 128], fp32, kind="ExternalInput")
    ident = nc.dram_tensor("ident", [128, 128], cdt, kind="ExternalInput")

    logits_out = nc.dram_tensor("logits_out", [1, VS], fp32, kind="ExternalOutput")
    h_out = nc.dram_tensor("h_out", [2, H], fp32, kind="ExternalOutput")
    attnw_out = nc.dram_tensor("attnw_out", [1, 128], fp32, kind="ExternalOutput")

    RG = [list(range(NC))]
    AF = mybir.ActivationFunctionType
    ALU = mybir.AluOpType
    AX = mybir.AxisListType

    with tile.TileContext(nc) as tc:
        with (
            tc.tile_pool(name="wpool", bufs=1) as wpool,       # resident weights
            tc.tile_pool(name="gpool", bufs=2) as gpool,       # GRU weights (2 slots)
            tc.tile_pool(name="spool", bufs=1) as spool,       # small resident tiles
            tc.tile_pool(name="stream", bufs=stream_bufs) as stream,  # out_w tiles
            tc.tile_pool(name="scratch", bufs=1) as scratch,   # chain temporaries
            tc.tile_pool(name="epool", bufs=3) as epool,       # exp throwaway
            tc.tile_pool(name="pchain", bufs=2, space="PSUM") as pchain,
            tc.tile_pool(name="ptr", bufs=1, space="PSUM") as ptr,
            tc.tile_pool(name="plog", bufs=4, space="PSUM") as plog,
            tc.tile_pool(name="dram", bufs=1, space="DRAM") as dram,
        ):
            sdma = nc.sync.dma_start
            gdma = nc.gpsimd.dma_start

            # Warmup AllGather: the 8 PJRT executions launch with tens of us
            # of skew; the first collective blocks until the slowest core
            # arrives.  Pay that cost NOW, overlapped with the weight DMA
            # flood, so the chain's real AllGathers run at their ~6us floor.
            wu_in = dram.tile([1, 1], fp32, tag="wu_in")
            wu_out = dram.tile([NC, 1], fp32, tag="wu_out")
            wu_sb = scratch.tile([1, 1], fp32, tag="wu_sb")
            nc.gpsimd.memset(wu_sb[:, :], 0.0)
            gdma(wu_in[:, :], wu_sb[:, :])
            nc.gpsimd.collective_compute(
                "AllGather", ALU.bypass, replica_groups=RG,
                ins=[wu_in[:, :].opt()], outs=[wu_out[:, :].opt()])

            def load(pool, src_ap, shape, dtype, tag):
                t = pool.tile(shape, dtype, tag=tag)
                sdma(t[:], src_ap)
                return t

            # ---------- resident small loads (chain-critical first) ----------
            ain_sb = load(spool, aincol[:, :], [128, 16], cdt, "ain")
            emb_sb = load(spool, embcol[:, :], [128, 8], cdt, "emb")
            h0p_sb = load(spool, h0pcol[:, :], [128, 8], cdt, "h0p")
            h1p_sb = load(spool, h1pcol[:, :], [128, 8], cdt, "h1p")
            h0ps_sb = load(spool, h0pshard[:, :], [1, 128], fp32, "h0ps")
            h1ps_sb = load(spool, h1pshard[:, :], [1, 128], fp32, "h1ps")
            id_sb = load(spool, ident[:, :], [128, 128], cdt, "ident")
            attnb_sb = load(spool, attnb[:, :], [1, 128], fp32, "attnb")
            combb_sb = load(spool, combb[:, :], [1, 128], fp32, "combb")

            r3 = lambda ap: ap.rearrange("(kt p) n -> p kt n", p=128)
            attnT_sb = load(wpool, r3(attnT[:, :]), [128, 16, 128], cdt, "attnT")
            enc_sb = load(wpool, enc[:, :], [128, H], cdt, "enc")
            combT_sb = load(wpool, r3(combT[:, :]), [128, 16, 128], cdt, "combT")
            grub_sb = {}
            for layer in (0, 1):
                for kind in ("ih", "hh"):
                    grub_sb[(layer, kind)] = load(
                        spool, grub[(layer, kind)][:, :], [1, 384], fp32,
                        f"gb{kind}{layer}")
            outb_sb = load(spool, outb[:, :], [1, VS], fp32, "outb")

            def load_gru(layer, kind):
                return load(gpool, r3(gruT[(layer, kind)][:, :]),
                            [128, 8, 384], cdt, "gruw")

            gih0_sb = load_gru(0, "ih")
            ghh0_sb = load_gru(0, "hh")

            # ---------- attention ----------
            at_ps = pchain.tile([128, 512], fp32, tag="chain")
            for kt in range(16):
                nc.tensor.matmul(at_ps[0:1, 0:128], ain_sb[:, kt:kt + 1],
                                 attnT_sb[:, kt, :],
                                 start=(kt == 0), stop=(kt == 15))
            atl_sb = scratch.tile([1, 128], fp32, tag="atl")
            nc.vector.tensor_add(atl_sb[:, :], at_ps[0:1, 0:128], attnb_sb[:, :])
            # softmax along the free axis
            atm = scratch.tile([1, 1], fp32, tag="atm")
            nc.vector.reduce_max(out=atm[:, :], in_=atl_sb[:, :], axis=AX.X)
            natm = scratch.tile([1, 1], fp32, tag="natm")
            nc.vector.tensor_scalar_mul(natm[:, :], atm[:, :], -1.0)
            ate = scratch.tile([1, 128], fp32, tag="ate")
            ats = scratch.tile([1, 1], fp32, tag="ats")
            nc.scalar.activation(ate[:, :], atl_sb[:, :], AF.Exp,
                                 bias=natm[:, :], scale=1.0,
                                 accum_out=ats[:, :])
            atr = scratch.tile([1, 1], fp32, tag="atr")
            nc.vector.reciprocal(atr[:, :], ats[:, :])
            attnw_sb = spool.tile([1, 128], fp32, tag="attnw")
            nc.vector.tensor_scalar_mul(attnw_sb[:, :], ate[:, :], atr[:, :])
            sdma(attnw_out[:, :], attnw_sb[:, :])

            # transpose attn weights row -> column (128,1)
            wcol_ps = ptr.tile([128, 512], cdt, tag="chainT")
            if cdt != fp32:
                attnw_c = scratch.tile([1, 128], cdt, tag="attnwc")
                nc.vector.tensor_copy(attnw_c[:, :], attnw_sb[:, :])
                nc.tensor.transpose(wcol_ps[:, 0:1], attnw_c[:, :],
                                    id_sb[0:1, 0:1])
            else:
                nc.tensor.transpose(wcol_ps[:, 0:1], attnw_sb[:, :],
                                    id_sb[0:1, 0:1])
            wcol_sb = spool.tile([128, 1], cdt, tag="wcol")
            nc.vector.tensor_copy(wcol_sb[:, :], wcol_ps[:, 0:1])

            # attn_applied in column layout: applied[m] = sum_l w[l] * enc[l, m]
            ap_ps = pchain.tile([128, 512], fp32, tag="chain")
            for t in range(8):
                nc.tensor.matmul(ap_ps[:, t:t + 1],
                                 enc_sb[:, t * 128:(t + 1) * 128],
                                 wcol_sb[:, :], start=True, stop=True)
            apcol_sb = spool.tile([128, 8], cdt, tag="apcol")
            nc.vector.tensor_copy(apcol_sb[:, :], ap_ps[:, 0:8])

            # ---------- attn_combine + relu ----------
            cb_ps = pchain.tile([128, 512], fp32, tag="chain")
            for kt in range(16):
                acol = emb_sb[:, kt:kt + 1] if kt < 8 else apcol_sb[:, kt - 8:kt - 7]
                nc.tensor.matmul(cb_ps[0:1, 0:128], acol, combT_sb[:, kt, :],
                                 start=(kt == 0), stop=(kt == 15))
            xsum_sb = scratch.tile([1, 128], fp32, tag="xsum")
            nc.vector.tensor_add(xsum_sb[:, :], cb_ps[0:1, 0:128], combb_sb[:, :])
            xsh_sb = spool.tile([1, 128], fp32, tag="xsh")
            nc.scalar.activation(xsh_sb[:, :], xsum_sb[:, :], AF.Relu)

            # ---------- AllGather helper ----------
            def allgather_shard(row_sb, tag):
                """row_sb (1,128) f32 -> full (8,128) DRAM + (128,8) col SBUF."""
                ag_in = dram.tile([1, 128], fp32, tag=f"{tag}_in")
                ag_out = dram.tile([NC, 128], fp32, tag=f"{tag}_out")
                gdma(ag_in[:, :], row_sb[:, :])
                nc.gpsimd.collective_compute(
                    "AllGather", ALU.bypass, replica_groups=RG,
                    ins=[ag_in[:, :].opt()], outs=[ag_out[:, :].opt()])
                colf = spool.tile([128, NC], fp32, tag=f"{tag}_colf")
                sdma(colf[:, :], ag_out[:, :].rearrange("t p -> p t"))
                if cdt == fp32:
                    return ag_out, colf
                col = spool.tile([128, NC], cdt, tag=f"{tag}_col")
                nc.vector.tensor_copy(col[:, :], colf[:, :])
                return ag_out, col

            x_ag, xcol_sb = allgather_shard(xsh_sb, "x")

            # ---------- GRU cell ----------
            def gru_cell(wih_sb, whh_sb, layer, xcol, hpcol, hpshard_sb, tag):
                # gh first: it depends only on host-shipped h_prev, so the PE
                # can run it while the AllGather feeding xcol is in flight.
                gi_ps = pchain.tile([128, 512], fp32, tag="chain")
                gh_ps = pchain.tile([128, 512], fp32, tag="chain")
                for kt in range(8):
                    nc.tensor.matmul(gh_ps[0:1, 0:384], hpcol[:, kt:kt + 1],
                                     whh_sb[:, kt, :],
                                     start=(kt == 0), stop=(kt == 7))
                for kt in range(8):
                    nc.tensor.matmul(gi_ps[0:1, 0:384], xcol[:, kt:kt + 1],
                                     wih_sb[:, kt, :],
                                     start=(kt == 0), stop=(kt == 7))
                gi = scratch.tile([1, 384], fp32, tag=f"gis{tag}")
                gh = scratch.tile([1, 384], fp32, tag=f"ghs{tag}")
                nc.vector.tensor_add(gi[:, :], gi_ps[0:1, 0:384],
                                     grub_sb[(layer, "ih")][:, :])
                nc.vector.tensor_add(gh[:, :], gh_ps[0:1, 0:384],
                                     grub_sb[(layer, "hh")][:, :])
                rz_in = scratch.tile([1, 256], fp32, tag=f"rzin{tag}")
                nc.vector.tensor_add(rz_in[:, :], gi[:, 0:256], gh[:, 0:256])
                rz = scratch.tile([1, 256], fp32, tag=f"rz{tag}")
                nc.scalar.activation(rz[:, :], rz_in[:, :], AF.Sigmoid)
                t1 = scratch.tile([1, 128], fp32, tag=f"t1{tag}")
                nc.vector.tensor_mul(t1[:, :], rz[:, 0:128], gh[:, 256:384])
                t2 = scratch.tile([1, 128], fp32, tag=f"t2{tag}")
                nc.vector.tensor_add(t2[:, :], gi[:, 256:384], t1[:, :])
                n_t = scratch.tile([1, 128], fp32, tag=f"n{tag}")
                nc.scalar.activation(n_t[:, :], t2[:, :], AF.Tanh)
                hmn = scratch.tile([1, 128], fp32, tag=f"hmn{tag}")
                nc.vector.tensor_sub(hmn[:, :], hpshard_sb[:, :], n_t[:, :])
                zh = scratch.tile([1, 128], fp32, tag=f"zh{tag}")
                nc.vector.tensor_mul(zh[:, :], rz[:, 128:256], hmn[:, :])
                hnew = spool.tile([1, 128], fp32, tag=f"hnew{tag}")
                nc.vector.tensor_add(hnew[:, :], n_t[:, :], zh[:, :])
                return hnew

            h0_sb = gru_cell(gih0_sb, ghh0_sb, 0, xcol_sb, h0p_sb, h0ps_sb, "L0")
            gih1_sb = load_gru(1, "ih")
            ghh1_sb = load_gru(1, "hh")
            h0_ag, h0col_sb = allgather_shard(h0_sb, "h0")
            h1_sb = gru_cell(gih1_sb, ghh1_sb, 1, h0col_sb, h1p_sb, h1ps_sb, "L1")
            h1_ag, h1col_sb = allgather_shard(h1_sb, "h1")
            if wdt != cdt:
                h1col_w = spool.tile([128, NC], wdt, tag="h1colw")
                nc.vector.tensor_copy(h1col_w[:, :], h1col_sb[:, :])
            else:
                h1col_w = h1col_sb

            # write full new_hidden (every core has it after the AllGathers)
            row_ap = lambda ag: ag[:, :].rearrange("(o t) p -> o (t p)", o=1)
            hrow = scratch.tile([1, H], fp32, tag="hrow")
            sdma(hrow[:, :], row_ap(h0_ag))
            sdma(h_out[0:1, :], hrow[:, :])
            hrow2 = scratch.tile([1, H], fp32, tag="hrow2")
            sdma(hrow2[:, :], row_ap(h1_ag))
            sdma(h_out[1:2, :], hrow2[:, :])

            # ---------- streamed logits matmul + fused exp/accum ----------
            logits_sb = spool.tile([1, VS], fp32, tag="logits")
            ssum_sb = spool.tile([1, len(NT_SIZES)], fp32, tag="ssum")
            ns = 0
            for nt, w in enumerate(NT_SIZES):
                wt = stream.tile([128, 8, 512], wdt, tag="wtile")
                sdma(wt[:, :, :w], wT[:, ns:ns + w].rearrange(
                    "(kt p) n -> p kt n", p=128))
                lg_ps = plog.tile([1, 512], fp32, tag="lgps")
                for kt in range(8):
                    nc.tensor.matmul(lg_ps[:, :w], h1col_w[:, kt:kt + 1],
                                     wt[:, kt, :w],
                                     start=(kt == 0), stop=(kt == 7))
                nc.vector.tensor_add(logits_sb[:, ns:ns + w], lg_ps[:, :w],
                                     outb_sb[:, ns:ns + w])
                esc = epool.tile([1, 512], fp32, tag="esc")
                nc.scalar.activation(esc[:, :w], logits_sb[:, ns:ns + w],
                                     AF.Exp, accum_out=ssum_sb[:, nt:nt + 1])
                ns += w

            # ---------- global logsumexp + final subtract ----------
            sloc = scratch.tile([1, 1], fp32, tag="sloc")
            nc.vector.reduce_sum(out=sloc[:, :], in_=ssum_sb[:, :], axis=AX.X)
            s_in = dram.tile([1, 1], fp32, tag="s_in")
            s_out = dram.tile([NC, 1], fp32, tag="s_out")
            gdma(s_in[:, :], sloc[:, :])
            nc.gpsimd.collective_compute(
                "AllGather", ALU.bypass, replica_groups=RG,
                ins=[s_in[:, :].opt()], outs=[s_out[:, :].opt()])
            srow = scratch.tile([1, NC], fp32, tag="srow")
            sdma(srow[:, :], s_out[:, :].rearrange("(o t) p -> o (t p)", o=1))
            sg = scratch.tile([1, 1], fp32, tag="sg")
            nc.vector.reduce_sum(out=sg[:, :], in_=srow[:, :], axis=AX.X)
            lse = scratch.tile([1, 1], fp32, tag="lse")
            nc.scalar.activation(lse[:, :], sg[:, :], AF.Ln)
            nlse = scratch.tile([1, 1], fp32, tag="nlse")
            nc.vector.tensor_scalar_mul(nlse[:, :], lse[:, :], -1.0)
            # split the (1, VS) subtract across DVE and ACT to halve the tail
            half = (VS // 2) // 512 * 512
            nc.vector.tensor_scalar_add(logits_sb[:, :half],
                                        logits_sb[:, :half], nlse[:, :])
            nc.scalar.activation(logits_sb[:, half:], logits_sb[:, half:],
                                 AF.Identity, bias=nlse[:, :], scale=1.0)
            sdma(logits_out[:, 0:half], logits_sb[:, 0:half])
            sdma(logits_out[:, half:], logits_sb[:, half:])

    nc.compile()
    return nc


def _prep_in_maps(inputs, chain_dtype="float32", wt_dtype="bfloat16"):
    """Shard + re-layout the full inputs into 8 per-core input dicts."""
    import concourse.mybir as mybir
    cnp = mybir.dt.np(getattr(mybir.dt, chain_dtype))
    wnp = mybir.dt.np(getattr(mybir.dt, wt_dtype))
    f32 = np.float32

    def c(a):
        return np.ascontiguousarray(np.asarray(a, dtype=f32).astype(cnp))

    def cf(a):
        return np.ascontiguousarray(np.asarray(a, dtype=f32))

    emb = np.asarray(inputs["emb"], f32)
    tok = int(np.asarray(inputs["input_ids"]).reshape(-1)[0])
    embedded = emb[tok]                                # (1024,) host gather (4KB)
    hidden = np.asarray(inputs["hidden"], f32)         # (2,1,1024)
    h0p, h1p = hidden[0, 0], hidden[1, 0]

    wT_full = np.zeros((H, NC * VS), f32)
    wT_full[:, :V] = np.asarray(inputs["out_w"], f32).T
    wT_full = wT_full.astype(wnp)
    outb_full = np.full((NC * VS,), PAD_BIAS, f32)
    outb_full[:V] = np.asarray(inputs["out_b"], f32)

    gruT = {}
    grub = {}
    for layer in (0, 1):
        for kind in ("ih", "hh"):
            gruT[(layer, kind)] = np.asarray(inputs[f"w_{kind}{layer}"], f32).T
            grub[(layer, kind)] = np.asarray(inputs[f"b_{kind}{layer}"], f32)
    combT = np.asarray(inputs["comb_w"], f32).T        # (2048, 1024)
    combb = np.asarray(inputs["comb_b"], f32)
    attnT = c(np.asarray(inputs["attn_w"], f32).T)     # (2048, 128)
    attnb = cf(np.asarray(inputs["attn_b"], f32)[None, :])
    encv = c(inputs["encoder_outputs"])                # (128, 1024)
    aincol = c(np.concatenate([embedded, h0p]).reshape(16, 128).T)
    embcol = c(embedded.reshape(8, 128).T)
    h0pcol = c(h0p.reshape(8, 128).T)
    h1pcol = c(h1p.reshape(8, 128).T)
    ident = c(np.eye(128, dtype=f32))

    in_maps = []
    for core in range(NC):
        s, e = core * 128, (core + 1) * 128
        m = {
            "wT": np.ascontiguousarray(wT_full[:, core * VS:(core + 1) * VS]),
            "outb": cf(outb_full[core * VS:(core + 1) * VS][None, :]),
            "combT": c(combT[:, s:e]),
            "combb": cf(combb[s:e][None, :]),
            "attnT": attnT,
            "attnb": attnb,
            "enc": encv,
            "aincol": aincol,
            "embcol": embcol,
            "h0pcol": h0pcol,
            "h1pcol": h1pcol,
            "h0pshard": cf(h0p[s:e][None, :]),
            "h1pshard": cf(h1p[s:e][None, :]),
            "ident": ident,
        }
        for layer in (0, 1):
            for kind in ("ih", "hh"):
                t = gruT[(layer, kind)]   # (1024, 3072)
                m[f"w{kind}{layer}T"] = c(np.concatenate(
                    [t[:, g * H + s:g * H + e] for g in range(3)], axis=1))
                b = grub[(layer, kind)]
                m[f"b{kind}{layer}"] = cf(np.concatenate(
                    [b[g * H + s:g * H + e] for g in range(3)])[None, :])
        in_maps.append(m)
    return in_maps


def _ensure_ntff_hook():
    """Wire up the axon NTFF profile hook (missing antenv.axon_hooks shim).

    Only used by the local test harness (KERNEL_TRACE=1); replicates
    trn_agent_boot.trn_boot._ntff_profile_via_ctypes.
    """
    import types
    import ctypes
    import contextlib

    try:
        from antenv.axon_hooks import get_axon_ntff_profile_hook
        if get_axon_ntff_profile_hook() is not None:
            return True
    except ImportError:
        pass

    so_path = "/opt/axon/libaxon_pjrt.so"
    if not os.path.exists(so_path):
        return False
    lib = ctypes.CDLL(so_path)
    if not hasattr(lib, "axon_start_nrt_profile"):
        return False
    lib.axon_start_nrt_profile.argtypes = [
        ctypes.POINTER(ctypes.c_int64), ctypes.c_size_t]
    lib.axon_start_nrt_profile.restype = ctypes.c_int64
    lib.axon_stop_nrt_profile.argtypes = [ctypes.c_char_p]
    lib.axon_stop_nrt_profile.restype = ctypes.c_int64

    @contextlib.contextmanager
    def _hook(output_dir, device_ids):
        import jax
        jax.devices()
        if device_ids:
            ids = (ctypes.c_int64 * len(device_ids))(*device_ids)
            rc = lib.axon_start_nrt_profile(ids, len(device_ids))
        else:
            rc = lib.axon_start_nrt_profile(None, 0)
        if rc != 0:
            raise RuntimeError(f"axon_start_nrt_profile rc={rc}")
        try:
            yield
        finally:
            n = lib.axon_stop_nrt_profile(str(output_dir).encode())
            print(f"profile: {n} file(s) written to {output_dir}",
                  file=sys.stderr)

    holder = {}
    mod = types.ModuleType("antenv.axon_hooks")
    mod.set_axon_ntff_profile_hook = lambda h: holder.__setitem__("h", h)
    mod.get_axon_ntff_profile_hook = lambda: holder.get("h")
    import antenv
    antenv.axon_hooks = mod
    sys.modules["antenv.axon_hooks"] = mod
    mod.set_axon_ntff_profile_hook(_hook)

    # avoid the network artifact upload in the trace post-processing
    from concourse import bass_utils
    bass_utils.upload_artifacts = lambda tmpdir: str(tmpdir)
    return True


def kernel(**inputs):
    global LAST_EXEC_NS, LAST_RESULTS
    from concourse import bass_utils

    chain_dtype = os.environ.get("KERNEL_CHAIN_DTYPE", "float32")
    wt_dtype = os.environ.get("KERNEL_WT_DTYPE", "bfloat16")
    key = ("prog", chain_dtype, wt_dtype)
    if key not in _CACHE:
        _CACHE[key] = _build_program(chain_dtype, wt_dtype)
    nc = _CACHE[key]

    in_maps = _prep_in_maps(inputs, chain_dtype, wt_dtype)
    trace = os.environ.get("KERNEL_TRACE", "0") == "1"
    if trace:
        trace = _ensure_ntff_hook()
    try:
        res = bass_utils.run_bass_kernel_spmd(
            nc, in_maps, core_ids=list(range(NC)), trace=trace)
    except Exception:
        if not trace:
            raise
        res = bass_utils.run_bass_kernel_spmd(
            nc, in_maps, core_ids=list(range(NC)), trace=False)
    LAST_EXEC_NS = res.exec_time_ns
    LAST_RESULTS = res

    logits = np.concatenate(
        [res.results[cc]["logits_out"][0] for cc in range(NC)])[:V][None, :]
    h = res.results[0]["h_out"].reshape(2, 1, H).astype(np.float32)
    attnw = res.results[0]["attnw_out"].reshape(1, 128).astype(np.float32)
    return logits.astype(np.float32), h, attnw
